# revision 18
# baseline (speedup 1.0000x reference)
"""Cached grouped-query multi-head attention on 8 Trainium2 cores.

Sharding: core c -> batch b = c//2, head-half = c%2 (8 of 16 heads, 2 of 4
KV groups per core). Wq/Wk column-parallel, Wo row-parallel; the two
partial Wo products per batch are summed on the host (the "all-reduce").

Device kernel (per core, fp32 data / float32r matmuls):
  x^T via PE transposes -> Q^T/K^T/V projections -> RoPE (head-dim stored
  even-dims-then-odd-dims so rotation halves are partition-contiguous;
  host permutes Wq/Wk columns accordingly) -> scores computed transposed
  [k, q] so softmax probs are already PV-ready -> exp (no max subtraction;
  scores are O(1)) -> multiplicative mask on partial tiles only ->
  PV (out^T layout) + all-ones matmul for the softmax denominator ->
  reciprocal scale -> row-parallel Wo -> partial [LQ, D] output.
"""

import math
import sys

import numpy as np

sys.path.insert(0, "/opt/trn_rl_repo")

B, LQ, D = 4, 1024, 2048
H, G = 16, 4
HD = 128            # head dim
GS = H // G         # heads per group
PAST = 1024
LK = PAST + LQ      # 2048
NCORES = 8
NH = 8              # local heads per core
NG = 2              # local groups per core
KSUB = D // 128     # 16 contraction subtiles over D
QC = LQ // 512      # 2 query chunks of 512
QS = LQ // 128      # 8 query subtiles of 128
KC = LK // 128      # 16 key chunks of 128
NCH = D // 512      # 4 output column chunks

_PERM = np.concatenate([np.arange(0, HD, 2), np.arange(1, HD, 2)])
_PROG_CACHE = {}


def _build_program(classes, n_part):
    """Build the per-core Bass/Tile program.

    classes[(qc, kc)] = ("full"|"skip"|"part", partial_idx_or_None),
    identical on every core (the mask is shared).
    """
    import concourse.bacc as bacc
    import concourse.mybir as mybir
    import concourse.tile as tile
    from concourse.masks import make_identity

    f32 = mybir.dt.float32
    f32r = mybir.dt.float32r
    AF = mybir.ActivationFunctionType
    OP = mybir.AluOpType

    nc = bacc.Bacc("TRN2", target_bir_lowering=False, debug=False,
                   num_devices=NCORES)

    x_d = nc.dram_tensor("x", [LQ, D], f32, kind="ExternalInput").ap()
    wq_d = nc.dram_tensor("wq", [D, NH * HD], f32r, kind="ExternalInput").ap()
    bq_d = nc.dram_tensor("bq", [NH, HD, 1], f32, kind="ExternalInput").ap()
    wk_d = nc.dram_tensor("wk", [D, NG * HD], f32r, kind="ExternalInput").ap()
    bk_d = nc.dram_tensor("bk", [NG, HD, 1], f32, kind="ExternalInput").ap()
    wv_d = nc.dram_tensor("wv", [D, NG * HD], f32r, kind="ExternalInput").ap()
    bv_d = nc.dram_tensor("bv", [1, NG * HD], f32, kind="ExternalInput").ap()
    pk_d = nc.dram_tensor("pk", [NG, PAST, HD], f32, kind="ExternalInput").ap()
    pv_d = nc.dram_tensor("pv", [NG, PAST, HD], f32r, kind="ExternalInput").ap()
    rot_d = nc.dram_tensor("rot", [LQ, HD // 2], f32, kind="ExternalInput").ap()
    wo_d = nc.dram_tensor("wo", [NH * HD, D], f32r, kind="ExternalInput").ap()
    bo_d = nc.dram_tensor("bo", [1, D], f32, kind="ExternalInput").ap()
    mp_d = None
    if n_part:
        mp_d = nc.dram_tensor("maskp", [n_part, 128, 512], f32,
                              kind="ExternalInput").ap()
    out_d = nc.dram_tensor("out", [LQ, D], f32, kind="ExternalOutput").ap()

    # active key chunks per query chunk: list of (kc, partial_idx|None)
    active = {qc: [(kc, classes[(qc, kc)][1])
                   for kc in range(KC) if classes[(qc, kc)][0] != "skip"]
              for qc in range(QC)}

    scl = 1.0 / math.sqrt(HD)

    with tile.TileContext(nc) as tc:
        with (
            tc.tile_pool(name="const", bufs=1) as const,
            tc.tile_pool(name="persist", bufs=1) as persist,
            tc.tile_pool(name="raw", bufs=3) as raw,
            tc.tile_pool(name="ropet", bufs=1) as ropetp,
        ):
            ident = const.tile([128, 128], f32)
            make_identity(nc, ident)
            ones_f = const.tile([128, 128], f32)
            nc.gpsimd.memset(ones_f, 1.0)
            ones_mat = const.tile([128, 128], f32r)
            nc.vector.tensor_copy(ones_mat, ones_f)

            QT = persist.tile([128, NH, LQ], f32r)     # roped Q^T (perm rows)
            KT = persist.tile([128, NG, LK], f32r)     # K^T cache (perm rows)
            V = [persist.tile([128, KC, HD], f32r, tag=f"v{g}", name=f"v{g}")
                 for g in range(NG)]

            # full-height rotary tables: rows 0:64 and 64:128 both hold the
            # 64 frequencies; ssgnF carries -sin on top, +sin on bottom, so
            #   roped = src*cosF + swap(src)*ssgnF
            # where swap exchanges the two partition halves (x1<->x2):
            #   top: x1*cos + x2*(-sin)   bot: x2*cos + x1*(+sin)
            cosF = const.tile([128, LQ], f32)
            ssgnF = const.tile([128, LQ], f32)

            def rope(src, dst):
                # src/dst [128, LQ]; rows 0:64 = even dims, 64:128 = odd
                swp = raw.tile([128, LQ], f32, tag="raw", name="swp")
                nc.sync.dma_start(swp[0:64], src[64:128])
                nc.sync.dma_start(swp[64:128], src[0:64])
                t = ropetp.tile([128, LQ], f32, tag="ropet")
                nc.vector.tensor_mul(t, swp, ssgnF)
                nc.vector.tensor_mul(dst, src, cosF)
                nc.vector.tensor_tensor(dst, dst, t, OP.add)

            # ---- phase 1: rotary tables + x^T ----
            with (
                tc.tile_pool(name="xt", bufs=1) as xtp,
                tc.tile_pool(name="pstp", bufs=3, space="PSUM") as pstp,
            ):
                xT = xtp.tile([128, KSUB, LQ], f32r)
                with tc.tile_pool(name="ph1", bufs=2) as ph1:
                    rotT = ropetp.tile([64, LQ], f32, tag="rotT")
                    for i in range(8):
                        rch = ph1.tile([128, 64], f32, tag="rot")
                        nc.sync.dma_start(rch,
                                          rot_d[i * 128:(i + 1) * 128, :])
                        ps = pstp.tile([128, 128], f32, tag="tp")
                        nc.tensor.transpose(ps[0:64, :], rch, ident)
                        nc.vector.tensor_copy(rotT[:, i * 128:(i + 1) * 128],
                                              ps[0:64, :])
                    # freq in [0, 2pi); Sin on ScalarE needs [-pi, pi]:
                    #   -sin(x) = sin(x - pi);  cos(x) = 1 - 2*sin^2(x/2)
                    negpi = const.tile([64, 1], f32)
                    nc.gpsimd.memset(negpi, -math.pi)
                    nc.scalar.activation(ssgnF[0:64], rotT, AF.Sin,
                                         bias=negpi)
                    s2 = ph1.tile([64, LQ], f32, tag="s2")
                    nc.scalar.activation(s2, rotT, AF.Sin, scale=0.5)
                    nc.vector.tensor_mul(s2, s2, s2)
                    nc.vector.tensor_scalar(cosF[0:64], s2, -2.0, 1.0,
                                            OP.mult, OP.add)
                    # replicate to the bottom half (sin with flipped sign)
                    nc.vector.tensor_scalar_mul(s2, ssgnF[0:64], -1.0)
                    nc.sync.dma_start(ssgnF[64:128], s2)
                    nc.sync.dma_start(cosF[64:128], cosF[0:64])

                    for i in range(QS):
                        for half in range(2):
                            xc = ph1.tile([128, 1024], f32, tag="xc")
                            nc.sync.dma_start(
                                xc, x_d[i * 128:(i + 1) * 128,
                                        half * 1024:(half + 1) * 1024])
                            for jj in range(8):
                                j = half * 8 + jj
                                ps = pstp.tile([128, 128], f32, tag="tp")
                                nc.tensor.transpose(
                                    ps, xc[:, jj * 128:(jj + 1) * 128], ident)
                                nc.vector.tensor_copy(
                                    xT[:, j, i * 128:(i + 1) * 128], ps)

                # ---- phase 2: projections (V, K + past KV, then Q) ----
                with tc.tile_pool(name="psproj", bufs=2,
                                  space="PSUM") as psproj:
                    # V = x @ Wv + bv  (natural [q, dv] layout)
                    with tc.tile_pool(name="wvp", bufs=1) as wvp:
                        wvt = wvp.tile([128, KSUB, NG * HD], f32r)
                        nc.sync.dma_start(
                            wvt, wv_d.rearrange("(ko ki) m -> ki ko m",
                                                ki=128))
                        bv_sb = const.tile([1, NG * HD], f32)
                        nc.sync.dma_start(bv_sb, bv_d)
                        bv_rep = const.tile([128, NG * HD], f32)
                        nc.gpsimd.partition_broadcast(bv_rep, bv_sb)
                        for qs in range(QS):
                            ps = psproj.tile([128, 512], f32)
                            for ko in range(KSUB):
                                nc.tensor.matmul(
                                    ps[:, :NG * HD],
                                    xT[:, ko,
                                       qs * 128:(qs + 1) * 128],
                                    wvt[:, ko, :],
                                    start=(ko == 0), stop=(ko == KSUB - 1))
                            for g in range(NG):
                                nc.vector.tensor_tensor(
                                    V[g][:, PAST // 128 + qs, :],
                                    ps[:, g * HD:(g + 1) * HD],
                                    bv_rep[:, g * HD:(g + 1) * HD], OP.add)

                    # K^T (roped) + past K^T (permuted transpose) + past V
                    with tc.tile_pool(name="wkp", bufs=2) as wkp:
                        for g in range(NG):
                            wkg = wkp.tile([128, KSUB, HD], f32r, tag="wk")
                            nc.sync.dma_start(
                                wkg, wk_d.rearrange("(ko ki) m -> ki ko m",
                                                    ki=128)
                                [:, :, g * HD:(g + 1) * HD])
                            bkt = const.tile([128, 1], f32, tag=f"bk{g}")
                            nc.sync.dma_start(bkt, bk_d[g])
                            kraw = raw.tile([128, LQ], f32, tag="raw")
                            for qc in range(QC):
                                ps = psproj.tile([128, 512], f32)
                                for ko in range(KSUB):
                                    nc.tensor.matmul(
                                        ps,
                                        wkg[:, ko, :],
                                        xT[:, ko, qc * 512:(qc + 1) * 512]
                                        ,
                                        start=(ko == 0),
                                        stop=(ko == KSUB - 1))
                                nc.vector.tensor_scalar_add(
                                    kraw[:, qc * 512:(qc + 1) * 512], ps, bkt)
                            rope(kraw, KT[:, g, PAST:])

                            # pk head-dim is pre-permuted on the host, so a
                            # plain transpose lands rows in rope layout
                            for kc in range(PAST // 128):
                                pkc = raw.tile([128, HD], f32, tag="pkc")
                                nc.sync.dma_start(
                                    pkc, pk_d[g, kc * 128:(kc + 1) * 128, :])
                                ps = pstp.tile([128, 128], f32, tag="tp")
                                nc.tensor.transpose(ps, pkc, ident)
                                nc.vector.tensor_copy(
                                    KT[:, g, kc * 128:(kc + 1) * 128], ps)
                                nc.sync.dma_start(
                                    V[g][:, kc, :],
                                    pv_d[g, kc * 128:(kc + 1) * 128, :])

                    # Q^T (roped), per head
                    with tc.tile_pool(name="wqp", bufs=2) as wqp:
                        for h in range(NH):
                            wqh = wqp.tile([128, KSUB, HD], f32r, tag="wq")
                            nc.sync.dma_start(
                                wqh, wq_d.rearrange("(ko ki) m -> ki ko m",
                                                    ki=128)
                                [:, :, h * HD:(h + 1) * HD])
                            bqt = const.tile([128, 1], f32, tag=f"bq{h}")
                            nc.sync.dma_start(bqt, bq_d[h])
                            qraw = raw.tile([128, LQ], f32, tag="raw")
                            for qc in range(QC):
                                ps = psproj.tile([128, 512], f32)
                                for ko in range(KSUB):
                                    nc.tensor.matmul(
                                        ps,
                                        wqh[:, ko, :],
                                        xT[:, ko, qc * 512:(qc + 1) * 512]
                                        ,
                                        start=(ko == 0),
                                        stop=(ko == KSUB - 1))
                                nc.vector.tensor_scalar_add(
                                    qraw[:, qc * 512:(qc + 1) * 512], ps, bqt)
                            rope(qraw, QT[:, h, :])

            # ---- phase 4: attention ----
            import contextlib
            ph45 = contextlib.ExitStack()
            attnp = ph45.enter_context(tc.tile_pool(name="attnp", bufs=1))
            attnT = attnp.tile([128, NH, LQ], f32r)
            with (
                tc.tile_pool(name="mpp", bufs=1) as mpp,
                tc.tile_pool(name="ptp", bufs=4) as ptp,
                tc.tile_pool(name="pssc", bufs=3, space="PSUM") as pssc,
                tc.tile_pool(name="pspv", bufs=2, space="PSUM") as pspv,
                tc.tile_pool(name="psdn", bufs=2, space="PSUM") as psdn,
            ):
                mp_sb = None
                if n_part:
                    mp_sb = mpp.tile([128, n_part, 512], f32)
                    for i in range(n_part):
                        nc.sync.dma_start(mp_sb[:, i, :], mp_d[i])

                for h in range(NH):
                    g = h // GS
                    for qc in range(QC):
                        act = active[qc]
                        n_act = len(act)
                        ps_pv = pspv.tile([128, 512], f32)
                        ps_dn = psdn.tile([128, 512], f32)
                        for i, (kc, midx) in enumerate(act):
                            ps_s = pssc.tile([128, 512], f32)
                            nc.tensor.matmul(
                                ps_s,
                                KT[:, g, kc * 128:(kc + 1) * 128]
                                ,
                                QT[:, h, qc * 512:(qc + 1) * 512]
                                ,
                                start=True, stop=True)
                            pt = ptp.tile([128, 512], f32r, tag="pt")
                            nc.scalar.activation(pt, ps_s, AF.Exp, scale=scl)
                            if midx is not None:
                                nc.vector.tensor_mul(pt, pt,
                                                     mp_sb[:, midx, :])
                            nc.tensor.matmul(
                                ps_pv, V[g][:, kc, :],
                                pt,
                                start=(i == 0), stop=(i == n_act - 1))
                            nc.tensor.matmul(
                                ps_dn, ones_mat,
                                pt,
                                start=(i == 0), stop=(i == n_act - 1))
                        rec = raw.tile([128, 512], f32, tag="rec")
                        nc.vector.reciprocal(rec, ps_dn)
                        nc.vector.tensor_mul(
                            attnT[:, h, qc * 512:(qc + 1) * 512], ps_pv, rec)

            # ---- phase 5: output projection ----
            with (
                tc.tile_pool(name="wop", bufs=2) as wop,
                tc.tile_pool(name="bop", bufs=1) as bop,
                tc.tile_pool(name="pso", bufs=3, space="PSUM") as pso,
            ):
                bo_sb = bop.tile([1, D], f32)
                nc.sync.dma_start(bo_sb, bo_d)
                bo_rep = bop.tile([128, D], f32)
                nc.gpsimd.partition_broadcast(bo_rep, bo_sb)
                for ncH in range(NCH):
                    wot = wop.tile([128, NH, 512], f32r, tag="wo")
                    nc.sync.dma_start(
                        wot, wo_d.rearrange("(ho hi) n -> hi ho n", hi=128)
                        [:, :, ncH * 512:(ncH + 1) * 512])
                    for qs in range(QS):
                        ps = pso.tile([128, 512], f32)
                        for h in range(NH):
                            nc.tensor.matmul(
                                ps,
                                attnT[:, h, qs * 128:(qs + 1) * 128]
                                ,
                                wot[:, h, :],
                                start=(h == 0), stop=(h == NH - 1))
                        ot = raw.tile([128, 512], f32, tag="ot")
                        nc.vector.tensor_tensor(
                            ot, ps, bo_rep[:, ncH * 512:(ncH + 1) * 512],
                            OP.add)
                        nc.sync.dma_start(
                            out_d[qs * 128:(qs + 1) * 128,
                                  ncH * 512:(ncH + 1) * 512], ot)
            ph45.close()

    nc.compile()
    return nc


def _classify_mask(mask):
    """Per-[128k x 512q] tile: full / skip / partial (+ fp32 tile data)."""
    mT = mask.T  # [LK, LQ]
    classes = {}
    partials = []
    for qc in range(QC):
        for kc in range(KC):
            t = mT[kc * 128:(kc + 1) * 128, qc * 512:(qc + 1) * 512]
            if t.all():
                classes[(qc, kc)] = ("full", None)
            elif not t.any():
                classes[(qc, kc)] = ("skip", None)
            else:
                classes[(qc, kc)] = ("part", len(partials))
                partials.append(np.ascontiguousarray(t, dtype=np.float32))
    maskp = np.stack(partials) if partials else None
    return classes, maskp


def _prep_in_maps(x, mask, rotary_freqs, past_k, past_v, Wq, bq, Wk, bk,
                  Wv, bv, Wo, bo, maskp, n_part):
    c32 = lambda a: np.ascontiguousarray(a, dtype=np.float32)
    in_maps = []
    for c in range(NCORES):
        b, half = c // 2, c % 2
        h0 = half * NH          # first global head
        g0 = half * NG          # first global group
        wq_c = np.concatenate(
            [Wq[:, (h0 + h) * HD + _PERM] for h in range(NH)], axis=1)
        bq_c = np.stack([bq[(h0 + h) * HD + _PERM] for h in range(NH)])
        wk_c = np.concatenate(
            [Wk[:, (g0 + g) * HD + _PERM] for g in range(NG)], axis=1)
        bk_c = np.stack([bk[(g0 + g) * HD + _PERM] for g in range(NG)])
        m = {
            "x": c32(x[b]),
            "wq": c32(wq_c),
            "bq": c32(bq_c[..., None]),
            "wk": c32(wk_c),
            "bk": c32(bk_c[..., None]),
            "wv": c32(Wv[:, g0 * HD:(g0 + NG) * HD]),
            "bv": c32(bv[g0 * HD:(g0 + NG) * HD][None, :]),
            "pk": c32(past_k[b, g0:g0 + NG][..., _PERM]),
            "pv": c32(past_v[b, g0:g0 + NG]),
            "rot": c32(rotary_freqs),
            "wo": c32(Wo[h0 * HD:(h0 + NH) * HD, :]),
            "bo": c32(bo[None, :] if half == 0 else np.zeros((1, D))),
        }
        if n_part:
            m["maskp"] = maskp
        in_maps.append(m)
    return in_maps


def _run(inputs, trace=False):
    from concourse import bass_utils

    classes, maskp = _classify_mask(np.asarray(inputs["mask"]))
    n_part = 0 if maskp is None else maskp.shape[0]
    key = tuple(sorted(classes.items()))
    if key not in _PROG_CACHE:
        _PROG_CACHE[key] = _build_program(classes, n_part)
    nc = _PROG_CACHE[key]

    in_maps = _prep_in_maps(
        np.asarray(inputs["x"]), np.asarray(inputs["mask"]),
        np.asarray(inputs["rotary_freqs"]), np.asarray(inputs["past_k"]),
        np.asarray(inputs["past_v"]), np.asarray(inputs["Wq"]),
        np.asarray(inputs["bq"]), np.asarray(inputs["Wk"]),
        np.asarray(inputs["bk"]), np.asarray(inputs["Wv"]),
        np.asarray(inputs["bv"]), np.asarray(inputs["Wo"]),
        np.asarray(inputs["bo"]), maskp, n_part)

    res = bass_utils.run_bass_kernel_spmd(
        nc, in_maps, list(range(NCORES)), trace=trace,
        trace_cores=list(range(NCORES)) if trace else None)

    out = np.empty((B, LQ, D), np.float32)
    for b in range(B):
        out[b] = res.results[2 * b]["out"] + res.results[2 * b + 1]["out"]
    return out, res


def kernel(**inputs) -> np.ndarray:
    out, _ = _run(inputs, trace=False)
    return out
